# revision 1
# baseline (speedup 1.0000x reference)
"""GQA (32q/8kv heads, RoPE, causal) TRN2 kernel v3.

Sharding: 8 cores = 2 batches x 4 kv-pairs. Core (b, kvp) owns batch b,
kv heads {2kvp, 2kvp+1}, q heads 8kvp..8kvp+7. Each core emits a partial
o [S, D] (bf16); host sums 4 partials per batch.

Precision: QKV projection in fp8e4 DoubleRow (0.5 cyc/row; contraction 2048
averages the quantization noise away). Attention core in bf16 (fp8 scores/
probs/V/at each cost 2-3% output error — the attention output is ~1/sqrt(n)
smaller than V so quantization noise does NOT average down relative to it).

Per-core pipeline:
  QKV proj (fp8 DR, token-layout) -> psum [128t, 1024]
  RoPE: 3 DVE ops (pair-swap with signed-sin table) -> qk bf16 [128, 640]
  PE transposes (bf16) -> qt [64, 8, S] / kt [64, 2, S]; V -> vsb (+ones col)
  Scores (bf16, trimmed to [qlo,512)): psc [128k, 512q]; diagonal gets a
    -240 triangular tile added via a second matmul into the same psum group
  exp on ACT (trimmed) -> probs bf16 [128, kb, 512]; Pool memsets [0,qlo)
  AV orientation-2: out = attn [128 tok, 65]: lhsT = probs block [128k, 128t]
    stationary, rhs = vsb [128k, 65] moving (65 free = full PE util); 4-head
    slabs [128, 4, 65] per psum bank; col 64 = denominator (per-partition!)
  Normalize: DVE recip [128,4] + one broadcast-free mul -> atq bf16 [128, 512]
  at transpose (PE) -> atT [128 chan, 4, S]; o-proj bf16 -> po [128, 512];
  Pool evicts to bf16, DMA out per token tile.
"""
import numpy as np
from contextlib import ExitStack

import concourse.bass as bass
from concourse import bacc
import concourse.mybir as mybir
import concourse.tile as tile
from concourse.bass_utils import run_bass_kernel_spmd
import ml_dtypes

F32 = mybir.dt.float32
BF16 = mybir.dt.bfloat16
FP8 = mybir.dt.float8e4
EXP = mybir.ActivationFunctionType.Exp
DR = mybir.MatmulPerfMode.DoubleRow

D = 2048
DH = 64
NCORES = 8
ROPE_BASE = 10000.0
MASKVAL = -240.0
EBIAS = -2.0     # probs = exp(0.125*scores - 2); cancels in normalization

_cached = {}


def build_nc(S=2048, dbg=False):
    NTT = S // 128
    NIT = S // 512
    KC = D // 128
    NH = 8
    nc = bacc.Bacc("TRN2", target_bir_lowering=False, debug=False)
    dbg_d = {}
    if dbg:
        dbg_d["d_qt"] = nc.declare_dram_parameter("d_qt", [64, NH, S], F32, isOutput=True)
        dbg_d["d_kt"] = nc.declare_dram_parameter("d_kt", [64, 2, S], F32, isOutput=True)
        dbg_d["d_vsb"] = nc.declare_dram_parameter("d_vsb", [128, 2, NTT, 65], F32, isOutput=True)
        dbg_d["d_at"] = nc.declare_dram_parameter("d_at", [128, 4, S], F32, isOutput=True)
        dbg_d["d_pb"] = nc.declare_dram_parameter("d_pb", [128, NTT, 512], F32, isOutput=True)
    xt8 = nc.declare_dram_parameter("xt8", [NTT, 128, KC, 2, 128], FP8, isOutput=False)
    wall8 = nc.declare_dram_parameter("wall8", [128, KC, 2, 768], FP8, isOutput=False)
    wotb = nc.declare_dram_parameter("wotb", [128, 4, D], BF16, isOutput=False)
    cosb = nc.declare_dram_parameter("cosb", [NTT, 128, 64], BF16, isOutput=False)
    sinsg = nc.declare_dram_parameter("sinsg", [NTT, 128, 64], BF16, isOutput=False)
    identf = nc.declare_dram_parameter("identf", [128, 128], F32, isOutput=False)
    trif = nc.declare_dram_parameter("trif", [128, 128], F32, isOutput=False)
    o = nc.declare_dram_parameter("o", [S, D], BF16, isOutput=True)

    with tile.TileContext(nc) as tc, ExitStack() as ctx:
        wp = ctx.enter_context(tc.tile_pool(name="weights", bufs=1))
        sp = ctx.enter_context(tc.tile_pool(name="state", bufs=1))
        xs = ctx.enter_context(tc.tile_pool(name="xstream", bufs=2))
        rp = ctx.enter_context(tc.tile_pool(name="ring", bufs=3))
        pr = ctx.enter_context(tc.tile_pool(name="probs", bufs=3))
        aq = ctx.enter_context(tc.tile_pool(name="atq", bufs=2))
        ob = ctx.enter_context(tc.tile_pool(name="osb", bufs=2))
        sm = ctx.enter_context(tc.tile_pool(name="small", bufs=2))

        # ---------- persistent weights / tables ----------
        wall = wp.tile([128, KC, 2, 768], FP8, tag="wall")
        wot = wp.tile([128, 4, D], BF16, tag="wot")
        cos_sb = wp.tile([128, NTT, 64], BF16, tag="cos")
        sin_sb = wp.tile([128, NTT, 64], BF16, tag="sin")
        idb = wp.tile([128, 128], BF16, tag="idb")
        trib = wp.tile([128, 128], BF16, tag="trib")
        nbias = wp.tile([128, 1], F32, tag="nbias")

        nc.sync.dma_start(wall[:], wall8[:, :, :, :])
        nc.sync.dma_start(wot[:], wotb[:, :, :])
        nc.sync.dma_start(cos_sb[:], cosb[:, :, :].rearrange("tt p c -> p tt c"))
        nc.sync.dma_start(sin_sb[:], sinsg[:, :, :].rearrange("tt p c -> p tt c"))
        idf_s = sm.tile([128, 128], F32, tag="idf")
        trf_s = sm.tile([128, 128], F32, tag="trf")
        nc.sync.dma_start(idf_s[:], identf[:, :])
        nc.sync.dma_start(trf_s[:], trif[:, :])
        nc.vector.tensor_copy(idb[:], idf_s[:])
        nc.vector.tensor_copy(trib[:], trf_s[:])
        nc.vector.memset(nbias[:], EBIAS)

        # ---------- per-core state ----------
        qt = sp.tile([64, NH, S], BF16, tag="qt")
        kt = sp.tile([64, 2, S], BF16, tag="kt")
        vsb = sp.tile([128, 2, NTT, 65], BF16, tag="vsb")
        atT = sp.tile([128, 4, S], BF16, tag="atT")
        nc.vector.memset(vsb[:, :, :, 64:65], 1.0)

        # ================= phase 1: QKV + rope + transposes =================
        pq_pool = ExitStack()
        pp_qkv = pq_pool.enter_context(tc.tile_pool(name="pqkv", bufs=2, space="PSUM"))
        pp_tr = pq_pool.enter_context(tc.tile_pool(name="ptr", bufs=2, space="PSUM"))
        pre_pool = ExitStack()
        pp_pre = pre_pool.enter_context(tc.tile_pool(name="presc", bufs=1, space="PSUM"))

        pbs = [None] * NH
        pre_pbs = {}
        prescored = set()

        def emit_scores(it, h, pool, scbufs, drain=None):
            kv = h // 4
            nkb = 4 * it + 4
            i0 = it * 512
            tag = "probs0" if (it == 0 and NTT >= 16) else "probs"
            pbufs = 8 if tag == "probs0" else 2
            pb = pr.tile([128, nkb, 512], BF16, tag=tag, bufs=pbufs,
                         name=f"pb{it}_{h}")
            pbs[h] = pb
            pre_pbs[(it, h)] = pb
            for kb in range(nkb):
                if drain and kb and kb % (max(2, nkb // 2)) == 0:
                    drain(1)
                diag = kb >= 4 * it
                qlo = (kb - 4 * it) * 128 if diag else 0
                psc = pool.tile([128, 512], F32, tag="sc", bufs=scbufs,
                                name=f"psc{it}_{h}_{kb}")
                nc.tensor.matmul(psc[:, qlo:512],
                                 kt[:, kv, kb * 128:(kb + 1) * 128],
                                 qt[:, h, i0 + qlo:i0 + 512],
                                 start=True, stop=not diag)
                if diag:
                    nc.tensor.matmul(psc[:, qlo:qlo + 128], idb[:], trib[:],
                                     start=False, stop=True)
                nc.scalar.activation(pb[:, kb, qlo:512], psc[:, qlo:512],
                                     EXP, scale=0.125, bias=nbias[:])
                if qlo:
                    nc.gpsimd.memset(pb[:, kb, 0:qlo], 0.0)

        def p1_tail(tt, qk8):
            tsl = slice(tt * 128, (tt + 1) * 128)
            qtr = pp_tr.tile([64, 8, 128], BF16, tag="qtr", name=f"qtr{tt}")
            ktr = pp_tr.tile([64, 8, 128], BF16, tag="qtr", name=f"ktr{tt}")
            ktr = ktr[:, 0:2, :]
            for h in range(8):
                nc.tensor.matmul(qtr[:, h, :], qk8[:, h * 64:(h + 1) * 64],
                                 idb[:], is_transpose=True,
                                 start=(h == 0), stop=(h == 7))
            for g in range(2):
                nc.tensor.matmul(ktr[:, g, :],
                                 qk8[:, 512 + g * 64:512 + (g + 1) * 64],
                                 idb[:], is_transpose=True,
                                 start=(g == 0), stop=(g == 1))
            nc.vector.tensor_copy(qt[:, :, tsl], qtr[:])
            nc.vector.tensor_copy(kt[:, :, tsl], ktr[:])

        prevq = []
        for tt in range(NTT):
            xtile = xs.tile([128, KC, 2, 128], FP8, tag="xt", name=f"xt{tt}")
            nc.sync.dma_start(xtile[:], xt8[tt])
            pq = pp_qkv.tile([128, 1024], F32, tag="pq", name=f"pq{tt}")
            # exact-ish fp8 hi/lo split: (xh+xl)(wh+wl) ~ xh wh + xl wh + xh wl
            # per chunk-pair: 3 DoubleRow matmuls (0.75x bf16 cost); xl*wl dropped
            for kp in range(KC // 2):
                xh = xtile[:, 2 * kp:2 * kp + 2, 0, :]
                xl = xtile[:, 2 * kp:2 * kp + 2, 1, :]
                for c0, c1 in ((0, 512), (512, 768)):
                    wh = wall[:, 2 * kp:2 * kp + 2, 0, c0:c1]
                    wl = wall[:, 2 * kp:2 * kp + 2, 1, c0:c1]
                    st = (kp == 0)
                    nc.tensor.matmul(pq[:, c0:c1], xh, wh, start=st, stop=False,
                                     perf_mode=DR)
                    nc.tensor.matmul(pq[:, c0:c1], xl, wh, start=False, stop=False,
                                     perf_mode=DR)
                    nc.tensor.matmul(pq[:, c0:c1], xh, wl, start=False,
                                     stop=(kp == KC // 2 - 1), perf_mode=DR)
            if len(prevq) >= 2:
                p1_tail(*prevq.pop(0))
            if 5 <= tt < 13 and NTT >= 16:
                emit_scores(0, tt - 5, pp_pre, 2)
                prescored.add((0, tt - 5))
            elif 13 <= tt < 15 and NTT >= 16:
                emit_scores(1, tt - 13, pp_pre, 2)
                prescored.add((1, tt - 13))
            # ACT evicts psum -> bf16 sbuf (GPSIMD cannot touch PSUM);
            # rope: op1 tmp = pairswap(qk)*sinsg (Pool); op2 t1 = qk*cos (Pool);
            # op3 qk8 = t1+tmp (DVE, bf16 2x)
            qkvb = rp.tile([128, 768], BF16, tag="qkvb")
            nc.vector.tensor_scalar_mul(qkvb[:], pq[:, 0:768], 1.0 / 64.0)
            qkv = qkvb[:, 0:640]
            swp = qkv.rearrange("p (h n two) -> p h n two", two=2, n=32)[..., ::-1]
            tmp = rp.tile([128, 640], BF16, tag="tmp")
            t1 = rp.tile([128, 640], BF16, tag="t1")
            qk8 = rp.tile([128, 640], BF16, tag="qk8")
            sin4 = sin_sb[:, tt, :].rearrange("p (one n two) -> p one n two",
                                              one=1, two=2).to_broadcast([128, 10, 32, 2])
            cos3 = cos_sb[:, tt, :].rearrange("p (one c) -> p one c",
                                              one=1).to_broadcast([128, 10, 64])
            nc.gpsimd.tensor_mul(tmp[:].rearrange("p (h n two) -> p h n two",
                                                  two=2, n=32),
                                 swp, sin4)
            nc.gpsimd.tensor_mul(t1[:].rearrange("p (h c) -> p h c", h=10),
                                 qkv.rearrange("p (h c) -> p h c", h=10), cos3)
            nc.vector.tensor_add(qk8[:], t1[:], tmp[:])
            nc.vector.tensor_copy(vsb[:, :, tt, 0:64],
                                  qkvb[:, 640:768].rearrange("p (kv c) -> p kv c",
                                                             kv=2))
            prevq.append((tt, qk8))
        for pv in prevq:
            p1_tail(*pv)
        pre_pool.close()
        pq_pool.close()

        # ================= phase 2+3: attention + o-proj =================
        pp_att = ctx.enter_context(tc.tile_pool(name="patt", bufs=1, space="PSUM"))
        avs = [None] * 2
        oproj_q = []

        for it in range(NIT):
            i0 = it * 512

            def emit_av(h):
                kv = h // 4
                pb = pbs[h]
                for tq in range(4):
                    tt = 4 * it + tq
                    if h % 2 == 0 and tq % 2 == 0:
                        avs[tq // 2] = pp_att.tile([128, 2, 2, 128], F32,
                                                   tag=f"avs{tq // 2}", bufs=1,
                                                   name=f"avs{it}_{h}_{tq}")
                    slab = avs[tq // 2]
                    for kb in range(tt + 1):
                        nc.tensor.matmul(slab[:, tq % 2, h % 2, 0:65],
                                         pb[:, kb, tq * 128:(tq + 1) * 128],
                                         vsb[:, kv, kb, :],
                                         start=(kb == 0), stop=(kb == tt),
                                         skip_group_check=True)

            def emit_norm(g):
                for tq in range(4):
                    tt = 4 * it + tq
                    tsl = slice(tt * 128, (tt + 1) * 128)
                    slab = avs[tq // 2][:, tq % 2]
                    rec = sm.tile([128, 2, 1], F32, tag="rec")
                    nc.vector.reciprocal(rec[:, :, 0], slab[:, :, 64])
                    atq = aq.tile([128, 2, 64], BF16, tag="atq",
                                  name=f"atq{it}_{g}_{tq}")
                    nc.vector.tensor_mul(atq[:], slab[:, :, 0:64],
                                         rec[:].to_broadcast([128, 2, 64]))
                    attr = pp_att.tile([128, 128], BF16, tag="sc", bufs=4,
                                       name=f"attr{it}_{g}_{tq}")
                    nc.tensor.matmul(attr[:],
                                     atq[:].rearrange("p f c -> p (f c)"),
                                     idb[:], is_transpose=True,
                                     start=True, stop=True)
                    nc.vector.tensor_copy(atT[:, g, tsl], attr[:])

            def drain_oproj(n):
                for _ in range(n):
                    if oproj_q:
                        oproj_q.pop(0)()

            def maybe_scores(h):
                if (it, h) in prescored:
                    pbs[h] = pre_pbs[(it, h)]
                    return
                emit_scores(it, h, pp_att, 4, drain=drain_oproj)

            maybe_scores(0)
            drain_oproj(2)
            for h in range(1, NH):
                maybe_scores(h)
                emit_av(h - 1)
                if h % 2 == 0:
                    emit_norm(h // 2 - 1)
                drain_oproj(2)
            emit_av(NH - 1)
            emit_norm(NH // 2 - 1)

            def queue_oproj(it_):
                state = {}
                for tq in range(4):
                    tt = 4 * it_ + tq
                    for nt in range(4):
                        def step(tt=tt, nt=nt):
                            tsl = slice(tt * 128, (tt + 1) * 128)
                            if nt == 0:
                                state[tt] = ob.tile([128, D], BF16, tag="osb",
                                                    name=f"osb{tt}")
                            osb = state[tt]
                            nsl = slice(nt * 512, (nt + 1) * 512)
                            po = pp_att.tile([128, 512], F32, tag="po", bufs=2,
                                             name=f"po{tt}_{nt}")
                            for c in range(4):
                                nc.tensor.matmul(po[:], atT[:, c, tsl],
                                                 wot[:, c, nsl],
                                                 start=(c == 0), stop=(c == 3))
                            nc.vector.tensor_copy(osb[:, nsl], po[:])
                            if nt == 3:
                                nc.sync.dma_start(o[tsl, :], osb[:])
                        oproj_q.append(step)
            queue_oproj(it)
        while oproj_q:
            oproj_q.pop(0)()

        if dbg:
            dsc = ctx.enter_context(tc.tile_pool(name="dsc", bufs=1))
            for nm, t in [("d_qt", qt[:]), ("d_kt", kt[:]), ("d_vsb", vsb[:]),
                          ("d_at", atT[:]), ("d_pb", pbs[0][:])]:
                f = dsc.tile(list(t.shape), F32, tag="f" + nm, name="f" + nm)
                nc.vector.tensor_copy(f[:], t)
                nc.sync.dma_start(dbg_d[nm][tuple(slice(None) for _ in t.shape)], f[:])
    nc.compile()
    return nc


# ====================== host side ======================

def _fp8(x):
    return np.asarray(x, np.float32).astype(ml_dtypes.float8_e4m3)


def host_inputs(x, Wq, Wk, Wv, Wo, S=2048):
    NTT = S // 128
    KC = D // 128
    inv = ROPE_BASE ** (-np.arange(0, DH, 2, dtype=np.float64) / DH)
    th = np.arange(S, dtype=np.float64)[:, None] * inv[None, :]
    cos1 = np.repeat(np.cos(th), 2, axis=1)
    sin1 = np.sin(th)
    sinsg1 = np.empty((S, 64))
    sinsg1[:, 0::2] = -sin1
    sinsg1[:, 1::2] = sin1
    cosb = cos1.reshape(NTT, 128, 64).astype(ml_dtypes.bfloat16)
    sing = sinsg1.reshape(NTT, 128, 64).astype(ml_dtypes.bfloat16)
    identf = np.eye(128, dtype=np.float32)
    p = np.arange(128)[:, None]
    q = np.arange(128)[None, :]
    trif = np.where(p <= q, 0.0, MASKVAL).astype(np.float32)

    in_maps = []
    for c in range(NCORES):
        b, kvp = c // 4, c % 4
        xb = np.asarray(x[b], np.float32)
        xh = _fp8(xb)
        xl = _fp8(xb - xh.astype(np.float32))
        xt = np.stack([xh, xl], axis=0).reshape(2, NTT, 128, KC, 128)
        xt8 = np.ascontiguousarray(xt.transpose(1, 4, 3, 0, 2))
        wq = Wq[512 * kvp:512 * (kvp + 1)]
        wk = Wk[128 * kvp:128 * (kvp + 1)]
        wv = Wv[128 * kvp:128 * (kvp + 1)]
        wall = np.concatenate([wq, wk, wv], axis=0) * 64.0
        wh = _fp8(wall)
        wl = _fp8(wall - wh.astype(np.float32))
        wall8 = np.ascontiguousarray(
            np.stack([wh, wl], axis=0).transpose(2, 0, 1)
            .reshape(KC, 128, 2, 768).transpose(1, 0, 2, 3))
        wotb = np.ascontiguousarray(
            Wo[:, 512 * kvp:512 * (kvp + 1)].astype(ml_dtypes.bfloat16)
            .T.reshape(4, 128, D).transpose(1, 0, 2))
        in_maps.append(dict(xt8=xt8, wall8=wall8, wotb=wotb, cosb=cosb,
                            sinsg=sing, identf=identf, trif=trif))
    return in_maps


def kernel(**inputs):
    x = np.asarray(inputs["x"], dtype=np.float32)
    Wq = np.asarray(inputs["Wq"], dtype=np.float32)
    Wk = np.asarray(inputs["Wk"], dtype=np.float32)
    Wv = np.asarray(inputs["Wv"], dtype=np.float32)
    Wo = np.asarray(inputs["Wo"], dtype=np.float32)
    B, S, _ = x.shape
    in_maps = host_inputs(x, Wq, Wk, Wv, Wo, S=S)
    if "nc" not in _cached:
        _cached["nc"] = build_nc(S=S)
    res = run_bass_kernel_spmd(_cached["nc"], in_maps, list(range(NCORES)))
    out = np.zeros((B, S, D), np.float64)
    for c, r in enumerate(res.results):
        out[c // 4] += np.asarray(r["o"], np.float32)
    return out.astype(np.float32)



# revision 24
# speedup vs baseline: 1.1436x; 1.1436x over previous
"""GQA (32q/8kv heads, RoPE, causal) TRN2 kernel v3.

Sharding: 8 cores = 2 batches x 4 kv-pairs. Core (b, kvp) owns batch b,
kv heads {2kvp, 2kvp+1}, q heads 8kvp..8kvp+7. Each core emits a partial
o [S, D] (bf16); host sums 4 partials per batch.

Precision: QKV projection in fp8e4 DoubleRow (0.5 cyc/row; contraction 2048
averages the quantization noise away). Attention core in bf16 (fp8 scores/
probs/V/at each cost 2-3% output error — the attention output is ~1/sqrt(n)
smaller than V so quantization noise does NOT average down relative to it).

Per-core pipeline:
  QKV proj (fp8 DR, token-layout) -> psum [128t, 1024]
  RoPE: 3 DVE ops (pair-swap with signed-sin table) -> qk bf16 [128, 640]
  PE transposes (bf16) -> qt [64, 8, S] / kt [64, 2, S]; V -> vsb (+ones col)
  Scores (bf16, trimmed to [qlo,512)): psc [128k, 512q]; diagonal gets a
    -240 triangular tile added via a second matmul into the same psum group
  exp on ACT (trimmed) -> probs bf16 [128, kb, 512]; Pool memsets [0,qlo)
  AV orientation-2: out = attn [128 tok, 65]: lhsT = probs block [128k, 128t]
    stationary, rhs = vsb [128k, 65] moving (65 free = full PE util); 4-head
    slabs [128, 4, 65] per psum bank; col 64 = denominator (per-partition!)
  Normalize: DVE recip [128,4] + one broadcast-free mul -> atq bf16 [128, 512]
  at transpose (PE) -> atT [128 chan, 4, S]; o-proj bf16 -> po [128, 512];
  Pool evicts to bf16, DMA out per token tile.
"""
import numpy as np
from contextlib import ExitStack

import concourse.bass as bass
from concourse import bacc
import concourse.mybir as mybir
import concourse.tile as tile
from concourse.bass_utils import run_bass_kernel_spmd
import ml_dtypes

F32 = mybir.dt.float32
BF16 = mybir.dt.bfloat16
FP8 = mybir.dt.float8e4
EXP = mybir.ActivationFunctionType.Exp
DR = mybir.MatmulPerfMode.DoubleRow

D = 2048
DH = 64
NCORES = 8
ROPE_BASE = 10000.0
MASKVAL = -240.0
EBIAS = -2.0     # probs = exp(0.125*scores - 2); cancels in normalization

_cached = {}


def build_nc(S=2048, dbg=False):
    NTT = S // 128
    NIT = S // 512
    KC = D // 128
    NH = 8
    nc = bacc.Bacc("TRN2", target_bir_lowering=False, debug=False)
    dbg_d = {}
    if dbg:
        dbg_d["d_qt"] = nc.declare_dram_parameter("d_qt", [64, NH, S], F32, isOutput=True)
        dbg_d["d_kt"] = nc.declare_dram_parameter("d_kt", [64, 2, S], F32, isOutput=True)
        dbg_d["d_vsb"] = nc.declare_dram_parameter("d_vsb", [128, 2, NTT, 65], F32, isOutput=True)
        dbg_d["d_at"] = nc.declare_dram_parameter("d_at", [128, 4, S], F32, isOutput=True)
        dbg_d["d_pb"] = nc.declare_dram_parameter("d_pb", [128, NTT, 512], F32, isOutput=True)
    xt8 = nc.declare_dram_parameter("xt8", [NTT, 128, KC, 2, 128], FP8, isOutput=False)
    wall8 = nc.declare_dram_parameter("wall8", [128, KC, 2, 768], FP8, isOutput=False)
    wothb = nc.declare_dram_parameter("wothb", [128, 4, D], FP8, isOutput=False)
    wotlb = nc.declare_dram_parameter("wotlb", [128, 4, D], FP8, isOutput=False)
    cosb = nc.declare_dram_parameter("cosb", [128, NTT, 64], BF16, isOutput=False)
    sinsg = nc.declare_dram_parameter("sinsg", [128, NTT, 64], BF16, isOutput=False)
    identf = nc.declare_dram_parameter("identf", [128, 128], F32, isOutput=False)
    trif = nc.declare_dram_parameter("trif", [128, 128], F32, isOutput=False)
    o = nc.declare_dram_parameter("o", [S, D], BF16, isOutput=True)

    with tile.TileContext(nc) as tc, ExitStack() as ctx:
        wp = ctx.enter_context(tc.tile_pool(name="weights", bufs=1))
        sp = ctx.enter_context(tc.tile_pool(name="state", bufs=1))
        xs = ctx.enter_context(tc.tile_pool(name="xstream", bufs=3))
        rp = ctx.enter_context(tc.tile_pool(name="ring", bufs=3))
        pr = ctx.enter_context(tc.tile_pool(name="probs", bufs=3))
        aq = ctx.enter_context(tc.tile_pool(name="atq", bufs=2))
        ob = ctx.enter_context(tc.tile_pool(name="osb", bufs=2))
        sm = ctx.enter_context(tc.tile_pool(name="small", bufs=2))

        # ---------- persistent weights / tables ----------
        wall = wp.tile([128, KC, 2, 768], FP8, tag="wall")
        woth = wp.tile([128, 4, D], FP8, tag="woth")
        wotl = wp.tile([128, 4, D], FP8, tag="wotl")
        cos_sb = wp.tile([128, NTT, 64], BF16, tag="cos")
        sin_sb = wp.tile([128, NTT, 64], BF16, tag="sin")
        idb = wp.tile([128, 128], BF16, tag="idb")
        trib = wp.tile([128, 128], BF16, tag="trib")
        nbias = wp.tile([128, 1], F32, tag="nbias")

        # DMA issue order == execution order (single queue): x tiles first so
        # QKV can start ~4us in, then tables (needed by tile-0 rope), then
        # wall in per-chunk-pair pieces (matmul kp waits only on its chunk),
        # wot split across the loop (first needed ~90us in).
        xtiles = {}

        def xfetch(tt):
            t = xs.tile([128, KC, 2, 128], FP8, tag="xt", name=f"xt{tt}")
            nc.sync.dma_start(t[:], xt8[tt])
            xtiles[tt] = t

        def wfetch(kp):
            nc.sync.dma_start(wall[:, 2 * kp:2 * kp + 2, :, :],
                              wall8[:, 2 * kp:2 * kp + 2, :, :])

        xfetch(0)
        wfetch(0)
        wfetch(1)
        xfetch(1)
        nc.sync.dma_start(cos_sb[:], cosb[:, :, :])
        nc.sync.dma_start(sin_sb[:], sinsg[:, :, :])
        wfetch(2)
        idf_s = sm.tile([128, 128], F32, tag="idf")
        trf_s = sm.tile([128, 128], F32, tag="trf")
        nc.sync.dma_start(idf_s[:], identf[:, :])
        nc.sync.dma_start(trf_s[:], trif[:, :])
        for kp in range(3, KC // 2):
            wfetch(kp)
        nc.vector.tensor_copy(idb[:], idf_s[:])
        nc.vector.tensor_copy(trib[:], trf_s[:])
        nc.vector.memset(nbias[:], EBIAS)

        # ---------- per-core state ----------
        qt = sp.tile([64, NH, S], BF16, tag="qt")
        kt = sp.tile([64, 2, S], BF16, tag="kt")
        vsb = sp.tile([128, 2, NTT, 65], BF16, tag="vsb")
        # at held as fp8 hi/lo (o-proj runs in fp8 DoubleRow); atq carries
        # 16*at (ones-col = 1/16 makes rec = 16/denom) so the fp8-hi of small
        # at entries stays in the normal range; o eviction divides by 16*64.
        ath = sp.tile([128, 4, S], FP8, tag="ath")
        atl = sp.tile([128, 4, S], FP8, tag="atl")
        nc.vector.memset(vsb[:, :, :, 64:65], 1.0 / 16.0)

        # ================= phase 1: QKV + rope + transposes =================
        pq_pool = ExitStack()
        pp_qkv = pq_pool.enter_context(tc.tile_pool(name="pqkv", bufs=2, space="PSUM"))
        pp_tr = pq_pool.enter_context(tc.tile_pool(name="ptr", bufs=2, space="PSUM"))
        pre_pool = ExitStack()
        pp_pre = pre_pool.enter_context(tc.tile_pool(name="presc", bufs=1, space="PSUM"))

        pbs = [None] * NH
        pre_pbs = {}
        prescored = set()

        def score_group_thunks(it, h, pool, scbufs, paired):
            """Allocate pb for (it, h), return one thunk per psum score group
            (off-diag pair or diag single). Executing a thunk emits the
            matmul(s) + exp for that group."""
            kv = h // 4
            nkb = 4 * it + 4
            i0 = it * 512
            tag = "probs0" if (it == 0 and NTT >= 16) else "probs"
            pbufs = 8 if tag == "probs0" else 2
            pb = pr.tile([128, nkb, 512], BF16, tag=tag, bufs=pbufs,
                         name=f"pb{it}_{h}")
            pbs[h] = pb
            pre_pbs[(it, h)] = pb
            ndiag = 4 * it
            if paired:
                groups = [(kb, kb + 1) for kb in range(0, ndiag, 2)]
                groups += [(kb,) for kb in range(ndiag, nkb)]
            else:
                groups = [(kb,) for kb in range(nkb)]

            def emit_group(grp):
                if len(grp) == 2:
                    # off-diagonal pair: 2 one-bank psum slots, one wide exp
                    p2 = pool.tile([128, 2, 512], F32, tag="sc2", bufs=scbufs,
                                   name=f"psc{it}_{h}_{grp[0]}p")
                    for j, kb in enumerate(grp):
                        nc.tensor.matmul(p2[:, j, :],
                                         kt[:, kv, kb * 128:(kb + 1) * 128],
                                         qt[:, h, i0:i0 + 512],
                                         start=True, stop=True)
                    nc.scalar.activation(pb[:, grp[0]:grp[0] + 2, :], p2[:],
                                         EXP, scale=0.125, bias=nbias[:])
                    return
                kb = grp[0]
                diag = kb >= ndiag
                qlo = (kb - ndiag) * 128 if diag else 0
                psc = pool.tile([128, 512], F32, tag="sc2", bufs=scbufs,
                                name=f"psc{it}_{h}_{kb}")
                nc.tensor.matmul(psc[:, qlo:512],
                                 kt[:, kv, kb * 128:(kb + 1) * 128],
                                 qt[:, h, i0 + qlo:i0 + 512],
                                 start=True, stop=not diag)
                if diag:
                    nc.tensor.matmul(psc[:, qlo:qlo + 128], idb[:], trib[:],
                                     start=False, stop=True)
                nc.scalar.activation(pb[:, kb, qlo:512], psc[:, qlo:512],
                                     EXP, scale=0.125, bias=nbias[:])
                if qlo:
                    nc.gpsimd.memset(pb[:, kb, 0:qlo], 0.0)

            return [lambda grp=grp: emit_group(grp) for grp in groups]

        def emit_scores(it, h, pool, scbufs, paired=False):
            for t in score_group_thunks(it, h, pool, scbufs, paired):
                t()

        def p1_tail(tt, qk8):
            tsl = slice(tt * 128, (tt + 1) * 128)
            qtr = pp_tr.tile([64, 8, 128], BF16, tag="qtr", name=f"qtr{tt}")
            ktr = pp_tr.tile([64, 8, 128], BF16, tag="qtr", name=f"ktr{tt}")
            ktr = ktr[:, 0:2, :]
            for h in range(8):
                nc.tensor.matmul(qtr[:, h, :], qk8[:, h * 64:(h + 1) * 64],
                                 idb[:], is_transpose=True,
                                 start=(h == 0), stop=(h == 7))
            for g in range(2):
                nc.tensor.matmul(ktr[:, g, :],
                                 qk8[:, 512 + g * 64:512 + (g + 1) * 64],
                                 idb[:], is_transpose=True,
                                 start=(g == 0), stop=(g == 1))
            nc.vector.tensor_copy(qt[:, :, tsl], qtr[:])
            nc.vector.tensor_copy(kt[:, :, tsl], ktr[:])

        prevq = []
        for tt in range(NTT):
            if tt + 2 < NTT:
                xfetch(tt + 2)
            if tt in (3, 5, 7, 9):
                w4 = tt // 2 - 1
                nc.sync.dma_start(woth[:, w4, :], wothb[:, w4, :])
            elif tt in (4, 6, 8, 10):
                w4 = tt // 2 - 2
                nc.sync.dma_start(wotl[:, w4, :], wotlb[:, w4, :])
            xtile = xtiles.pop(tt)
            pq = pp_qkv.tile([128, 1024], F32, tag="pq", name=f"pq{tt}")
            # exact-ish fp8 hi/lo split: (xh+xl)(wh+wl) ~ xh wh + xl wh + xh wl
            # per chunk-pair: 3 DoubleRow matmuls (0.75x bf16 cost); xl*wl dropped
            for kp in range(KC // 2):
                xh = xtile[:, 2 * kp:2 * kp + 2, 0, :]
                xl = xtile[:, 2 * kp:2 * kp + 2, 1, :]
                for c0, c1 in ((0, 512), (512, 768)):
                    wh = wall[:, 2 * kp:2 * kp + 2, 0, c0:c1]
                    wl = wall[:, 2 * kp:2 * kp + 2, 1, c0:c1]
                    st = (kp == 0)
                    nc.tensor.matmul(pq[:, c0:c1], xh, wh, start=st, stop=False,
                                     perf_mode=DR)
                    nc.tensor.matmul(pq[:, c0:c1], xl, wh, start=False, stop=False,
                                     perf_mode=DR)
                    nc.tensor.matmul(pq[:, c0:c1], xh, wl, start=False,
                                     stop=(kp == KC // 2 - 1), perf_mode=DR)
            if len(prevq) >= 2:
                p1_tail(*prevq.pop(0))
            if 5 <= tt < 13 and NTT >= 16:
                emit_scores(0, tt - 5, pp_pre, 2)
                prescored.add((0, tt - 5))
            elif 13 <= tt < 15 and NTT >= 16:
                emit_scores(1, tt - 13, pp_pre, 2)
                prescored.add((1, tt - 13))
            # ACT evicts psum -> bf16 sbuf (GPSIMD cannot touch PSUM);
            # rope: op1 tmp = pairswap(qk)*sinsg (Pool); op2 t1 = qk*cos (Pool);
            # op3 qk8 = t1+tmp (DVE, bf16 2x)
            qkvb = rp.tile([128, 768], BF16, tag="qkvb")
            nc.scalar.activation(qkvb[:], pq[:, 0:768],
                                 mybir.ActivationFunctionType.Copy,
                                 scale=1.0 / 64.0)
            qkv = qkvb[:, 0:640]
            swp = qkv.rearrange("p (h n two) -> p h n two", two=2, n=32)[..., ::-1]
            tmp = rp.tile([128, 640], BF16, tag="tmp")
            t1 = rp.tile([128, 640], BF16, tag="t1")
            qk8 = rp.tile([128, 640], BF16, tag="qk8")
            sin4 = sin_sb[:, tt, :].rearrange("p (one n two) -> p one n two",
                                              one=1, two=2).to_broadcast([128, 10, 32, 2])
            cos3 = cos_sb[:, tt, :].rearrange("p (one c) -> p one c",
                                              one=1).to_broadcast([128, 10, 64])
            nc.gpsimd.tensor_mul(tmp[:].rearrange("p (h n two) -> p h n two",
                                                  two=2, n=32),
                                 swp, sin4)
            nc.gpsimd.tensor_mul(t1[:].rearrange("p (h c) -> p h c", h=10),
                                 qkv.rearrange("p (h c) -> p h c", h=10), cos3)
            nc.vector.tensor_add(qk8[:], t1[:], tmp[:])
            nc.vector.tensor_copy(vsb[:, :, tt, 0:64],
                                  qkvb[:, 640:768].rearrange("p (kv c) -> p kv c",
                                                             kv=2))
            prevq.append((tt, qk8))
        for pv in prevq:
            p1_tail(*pv)
        pre_pool.close()
        pq_pool.close()

        # ================= phase 2+3: attention + o-proj =================
        pp_att = ctx.enter_context(tc.tile_pool(name="patt", bufs=1, space="PSUM"))
        avs = [None] * 2
        oproj_q = []

        def drain_thunk():
            if oproj_q:
                oproj_q.pop(0)()

        carry = [drain_thunk, drain_thunk]
        for it in range(NIT):
            i0 = it * 512

            def av_thunks(it, h):
                kv = h // 4
                pb = pbs[h]
                out = []
                for tq in range(4):
                    def tqf(tq=tq, h=h, kv=kv, pb=pb, it=it):
                        tt = 4 * it + tq
                        if h % 2 == 0 and tq % 2 == 0:
                            avs[tq // 2] = pp_att.tile([128, 2, 2, 128], F32,
                                                       tag=f"avs{tq // 2}",
                                                       bufs=1,
                                                       name=f"avs{it}_{h}_{tq}")
                        slab = avs[tq // 2]
                        for kb in range(tt + 1):
                            nc.tensor.matmul(slab[:, tq % 2, h % 2, 0:65],
                                             pb[:, kb, tq * 128:(tq + 1) * 128],
                                             vsb[:, kv, kb, :],
                                             start=(kb == 0), stop=(kb == tt),
                                             skip_group_check=True)
                    out.append(tqf)
                return out

            def norm_thunks(it, g):
                out = []
                for tq in range(4):
                    def tqf(tq=tq, g=g, it=it):
                        tt = 4 * it + tq
                        tsl = slice(tt * 128, (tt + 1) * 128)
                        slab = avs[tq // 2][:, tq % 2]
                        rec = sm.tile([128, 2, 1], F32, tag="rec")
                        nc.vector.reciprocal(rec[:, :, 0], slab[:, :, 64])
                        atq = aq.tile([128, 2, 64], BF16, tag="atq",
                                      name=f"atq{it}_{g}_{tq}")
                        nc.vector.tensor_mul(atq[:], slab[:, :, 0:64],
                                             rec[:].to_broadcast([128, 2, 64]))
                        attr = pp_att.tile([128, 128], BF16, tag="po", bufs=2,
                                           name=f"attr{it}_{g}_{tq}")
                        nc.tensor.matmul(attr[:],
                                         atq[:].rearrange("p f c -> p (f c)"),
                                         idb[:], is_transpose=True,
                                         start=True, stop=True)
                        nc.vector.tensor_copy(atT[:, g, tsl], attr[:])
                    out.append(tqf)
                return out

            def queue_oproj_thunk(it_):
                def qt():
                    state = {}
                    for tq in range(4):
                        tt = 4 * it_ + tq
                        for nt in range(4):
                            def step(tt=tt, nt=nt):
                                tsl = slice(tt * 128, (tt + 1) * 128)
                                if nt == 0:
                                    state[tt] = ob.tile([128, D], BF16,
                                                        tag="osb",
                                                        name=f"osb{tt}")
                                osb = state[tt]
                                nsl = slice(nt * 512, (nt + 1) * 512)
                                po = pp_att.tile([128, 512], F32, tag="po",
                                                 bufs=2, name=f"po{tt}_{nt}")
                                for c in range(4):
                                    nc.tensor.matmul(po[:], atT[:, c, tsl],
                                                     wot[:, c, nsl],
                                                     start=(c == 0),
                                                     stop=(c == 3))
                                nc.vector.tensor_copy(osb[:, nsl], po[:])
                                if nt == 3:
                                    nc.sync.dma_start(o[tsl, :], osb[:])
                            oproj_q.append(step)
                return qt

            for h in range(NH):
                # filler: PE work whose results ACT doesn't gate — runs
                # between score groups so exp (2.3x slower than the score
                # matmul) keeps up without stalling PE on the psum ring.
                filler = list(carry)
                carry = []
                if h > 0:
                    filler += av_thunks(it, h - 1)
                    filler += [drain_thunk]
                    if h % 2 == 0:
                        filler += norm_thunks(it, h // 2 - 1)
                    filler += [drain_thunk]
                if (it, h) in prescored:
                    pbs[h] = pre_pbs[(it, h)]
                    sc = []
                else:
                    sc = score_group_thunks(it, h, pp_att, 2, paired=True)
                # riffle: 1 score group, 1 filler item, ...
                n = max(len(sc), len(filler))
                for i in range(n):
                    if i < len(sc):
                        sc[i]()
                    if i < len(filler):
                        filler[i]()
            carry = av_thunks(it, NH - 1) + [drain_thunk]
            carry += norm_thunks(it, NH // 2 - 1)
            carry += [queue_oproj_thunk(it), drain_thunk, drain_thunk]
        for t in carry:
            t()
        while oproj_q:
            oproj_q.pop(0)()

        if dbg:
            dsc = ctx.enter_context(tc.tile_pool(name="dsc", bufs=1))
            for nm, t in [("d_qt", qt[:]), ("d_kt", kt[:]), ("d_vsb", vsb[:]),
                          ("d_at", atT[:]), ("d_pb", pbs[0][:])]:
                f = dsc.tile(list(t.shape), F32, tag="f" + nm, name="f" + nm)
                nc.vector.tensor_copy(f[:], t)
                nc.sync.dma_start(dbg_d[nm][tuple(slice(None) for _ in t.shape)], f[:])
    nc.compile()
    return nc


# ====================== host side ======================

def _fp8(x):
    return np.asarray(x, np.float32).astype(ml_dtypes.float8_e4m3)


def host_inputs(x, Wq, Wk, Wv, Wo, S=2048):
    NTT = S // 128
    KC = D // 128
    inv = ROPE_BASE ** (-np.arange(0, DH, 2, dtype=np.float64) / DH)
    th = np.arange(S, dtype=np.float64)[:, None] * inv[None, :]
    cos1 = np.repeat(np.cos(th), 2, axis=1)
    sin1 = np.sin(th)
    sinsg1 = np.empty((S, 64))
    sinsg1[:, 0::2] = -sin1
    sinsg1[:, 1::2] = sin1
    cosb = np.ascontiguousarray(
        cos1.reshape(NTT, 128, 64).transpose(1, 0, 2)).astype(ml_dtypes.bfloat16)
    sing = np.ascontiguousarray(
        sinsg1.reshape(NTT, 128, 64).transpose(1, 0, 2)).astype(ml_dtypes.bfloat16)
    identf = np.eye(128, dtype=np.float32)
    p = np.arange(128)[:, None]
    q = np.arange(128)[None, :]
    trif = np.where(p <= q, 0.0, MASKVAL).astype(np.float32)

    in_maps = []
    for c in range(NCORES):
        b, kvp = c // 4, c % 4
        xb = np.asarray(x[b], np.float32)
        xh = _fp8(xb)
        xl = _fp8(xb - xh.astype(np.float32))
        xt = np.stack([xh, xl], axis=0).reshape(2, NTT, 128, KC, 128)
        xt8 = np.ascontiguousarray(xt.transpose(1, 4, 3, 0, 2))
        wq = Wq[512 * kvp:512 * (kvp + 1)]
        wk = Wk[128 * kvp:128 * (kvp + 1)]
        wv = Wv[128 * kvp:128 * (kvp + 1)]
        wall = np.concatenate([wq, wk, wv], axis=0) * 64.0
        wh = _fp8(wall)
        wl = _fp8(wall - wh.astype(np.float32))
        wall8 = np.ascontiguousarray(
            np.stack([wh, wl], axis=0).transpose(2, 0, 1)
            .reshape(KC, 128, 2, 768).transpose(1, 0, 2, 3))
        wotb = np.ascontiguousarray(
            Wo[:, 512 * kvp:512 * (kvp + 1)].astype(ml_dtypes.bfloat16)
            .T.reshape(4, 128, D).transpose(1, 0, 2))
        in_maps.append(dict(xt8=xt8, wall8=wall8, wotb=wotb, cosb=cosb,
                            sinsg=sing, identf=identf, trif=trif))
    return in_maps


def kernel(**inputs):
    x = np.asarray(inputs["x"], dtype=np.float32)
    Wq = np.asarray(inputs["Wq"], dtype=np.float32)
    Wk = np.asarray(inputs["Wk"], dtype=np.float32)
    Wv = np.asarray(inputs["Wv"], dtype=np.float32)
    Wo = np.asarray(inputs["Wo"], dtype=np.float32)
    B, S, _ = x.shape
    in_maps = host_inputs(x, Wq, Wk, Wv, Wo, S=S)
    if "nc" not in _cached:
        _cached["nc"] = build_nc(S=S)
    res = run_bass_kernel_spmd(_cached["nc"], in_maps, list(range(NCORES)))
    out = np.zeros((B, S, D), np.float64)
    for c, r in enumerate(res.results):
        out[c // 4] += np.asarray(r["o"], np.float32)
    return out.astype(np.float32)



# revision 61
# speedup vs baseline: 1.1529x; 1.0081x over previous
"""GQA (32q/8kv heads, RoPE, causal) TRN2 kernel v3.

Sharding: 8 cores = 2 batches x 4 kv-pairs. Core (b, kvp) owns batch b,
kv heads {2kvp, 2kvp+1}, q heads 8kvp..8kvp+7. Each core emits a partial
o [S, D] (bf16); host sums 4 partials per batch.

Precision: QKV projection in fp8e4 DoubleRow (0.5 cyc/row; contraction 2048
averages the quantization noise away). Attention core in bf16 (fp8 scores/
probs/V/at each cost 2-3% output error — the attention output is ~1/sqrt(n)
smaller than V so quantization noise does NOT average down relative to it).

Per-core pipeline:
  QKV proj (fp8 DR, token-layout) -> psum [128t, 1024]
  RoPE: 3 DVE ops (pair-swap with signed-sin table) -> qk bf16 [128, 640]
  PE transposes (bf16) -> qt [64, 8, S] / kt [64, 2, S]; V -> vsb (+ones col)
  Scores (bf16, trimmed to [qlo,512)): psc [128k, 512q]; diagonal gets a
    -240 triangular tile added via a second matmul into the same psum group
  exp on ACT (trimmed) -> probs bf16 [128, kb, 512]; Pool memsets [0,qlo)
  AV orientation-2: out = attn [128 tok, 65]: lhsT = probs block [128k, 128t]
    stationary, rhs = vsb [128k, 65] moving (65 free = full PE util); 4-head
    slabs [128, 4, 65] per psum bank; col 64 = denominator (per-partition!)
  Normalize: DVE recip [128,4] + one broadcast-free mul -> atq bf16 [128, 512]
  at transpose (PE) -> atT [128 chan, 4, S]; o-proj bf16 -> po [128, 512];
  Pool evicts to bf16, DMA out per token tile.
"""
import numpy as np
from contextlib import ExitStack

import concourse.bass as bass
from concourse import bacc
import concourse.mybir as mybir
import concourse.tile as tile
from concourse.bass_utils import run_bass_kernel_spmd
import ml_dtypes

F32 = mybir.dt.float32
BF16 = mybir.dt.bfloat16
FP8 = mybir.dt.float8e4
EXP = mybir.ActivationFunctionType.Exp
DR = mybir.MatmulPerfMode.DoubleRow

D = 2048
DH = 64
NCORES = 8
ROPE_BASE = 10000.0
MASKVAL = -240.0
EBIAS = -2.0     # probs = exp(0.125*scores - 2); cancels in normalization

_cached = {}


def build_nc(S=2048, dbg=False):
    NTT = S // 128
    NIT = S // 512
    KC = D // 128
    NH = 8
    nc = bacc.Bacc("TRN2", target_bir_lowering=False, debug=False)
    dbg_d = {}
    if dbg:
        dbg_d["d_qt"] = nc.declare_dram_parameter("d_qt", [64, NH, S], F32, isOutput=True)
        dbg_d["d_kt"] = nc.declare_dram_parameter("d_kt", [64, 2, S], F32, isOutput=True)
        dbg_d["d_vsb"] = nc.declare_dram_parameter("d_vsb", [128, 2, NTT, 65], F32, isOutput=True)
        dbg_d["d_at"] = nc.declare_dram_parameter("d_at", [128, 4, S], F32, isOutput=True)
        dbg_d["d_pb"] = nc.declare_dram_parameter("d_pb", [128, NTT, 512], F32, isOutput=True)
    xt8 = nc.declare_dram_parameter("xt8", [NTT, 128, KC, 2, 128], FP8, isOutput=False)
    wall8 = nc.declare_dram_parameter("wall8", [128, KC, 2, 768], FP8, isOutput=False)
    wothb = nc.declare_dram_parameter("wothb", [128, 4, D], FP8, isOutput=False)
    wotlb = nc.declare_dram_parameter("wotlb", [128, 4, D], FP8, isOutput=False)
    cosb = nc.declare_dram_parameter("cosb", [128, NTT, 64], BF16, isOutput=False)
    sinsg = nc.declare_dram_parameter("sinsg", [128, NTT, 64], BF16, isOutput=False)
    identf = nc.declare_dram_parameter("identf", [128, 128], F32, isOutput=False)
    trif = nc.declare_dram_parameter("trif", [128, 128], F32, isOutput=False)
    o = nc.declare_dram_parameter("o", [S, D], BF16, isOutput=True)

    with tile.TileContext(nc) as tc, ExitStack() as ctx:
        wp = ctx.enter_context(tc.tile_pool(name="weights", bufs=1))
        sp = ctx.enter_context(tc.tile_pool(name="state", bufs=1))
        xs = ctx.enter_context(tc.tile_pool(name="xstream", bufs=3))
        rp = ctx.enter_context(tc.tile_pool(name="ring", bufs=2))
        pr = ctx.enter_context(tc.tile_pool(name="probs", bufs=3))
        aq = ctx.enter_context(tc.tile_pool(name="atq", bufs=2))
        ob = ctx.enter_context(tc.tile_pool(name="osb", bufs=2))
        sm = ctx.enter_context(tc.tile_pool(name="small", bufs=2))

        # ---------- persistent weights / tables ----------
        wall = wp.tile([128, KC, 2, 768], FP8, tag="wall")
        woth = wp.tile([128, 4, D], FP8, tag="woth")
        wotl = wp.tile([128, 4, D], FP8, tag="wotl")
        cos_sb = wp.tile([128, NTT, 64], BF16, tag="cos")
        sin_sb = wp.tile([128, NTT, 64], BF16, tag="sin")
        idb = wp.tile([128, 128], BF16, tag="idb")
        trib = wp.tile([128, 128], BF16, tag="trib")
        nbias = wp.tile([128, 1], F32, tag="nbias")

        # DMA issue order == execution order (single queue): x tiles first so
        # QKV can start ~4us in, then tables (needed by tile-0 rope), then
        # wall in per-chunk-pair pieces (matmul kp waits only on its chunk),
        # wot split across the loop (first needed ~90us in).
        xtiles = {}

        def xfetch(tt):
            t = xs.tile([128, KC, 2, 128], FP8, tag="xt", name=f"xt{tt}")
            nc.sync.dma_start(t[:], xt8[tt])
            xtiles[tt] = t

        def wfetch(kp):
            nc.sync.dma_start(wall[:, 2 * kp:2 * kp + 2, :, :],
                              wall8[:, 2 * kp:2 * kp + 2, :, :])

        # tile 0 in two pieces so the first matmul starts after ~0.4us of x
        xt0 = xs.tile([128, KC, 2, 128], FP8, tag="xt", name="xt0")
        xtiles[0] = xt0
        nc.sync.dma_start(xt0[:, 0:4], xt8[0][:, 0:4])
        wfetch(0)
        nc.sync.dma_start(xt0[:, 4:KC], xt8[0][:, 4:KC])
        wfetch(1)
        xfetch(1)
        nc.sync.dma_start(cos_sb[:], cosb[:, :, :])
        nc.sync.dma_start(sin_sb[:], sinsg[:, :, :])
        wfetch(2)
        idf_s = sm.tile([128, 128], F32, tag="idf")
        trf_s = sm.tile([128, 128], F32, tag="trf")
        nc.sync.dma_start(idf_s[:], identf[:, :])
        nc.sync.dma_start(trf_s[:], trif[:, :])
        for kp in range(3, KC // 2):
            wfetch(kp)
        nc.vector.tensor_copy(idb[:], idf_s[:])
        nc.vector.tensor_copy(trib[:], trf_s[:])
        nc.vector.memset(nbias[:], EBIAS)

        # ---------- per-core state ----------
        qt = sp.tile([64, NH, S], BF16, tag="qt")
        kt = sp.tile([64, 2, S], BF16, tag="kt")
        vsb = sp.tile([128, 2, NTT, 65], BF16, tag="vsb")
        # at held as fp8 hi/lo (o-proj runs in fp8 DoubleRow); atq carries
        # 16*at (ones-col = 1/16 makes rec = 16/denom) so the fp8-hi of small
        # at entries stays in the normal range; o eviction divides by 16*64.
        ath = sp.tile([128, 4, S], FP8, tag="ath")
        atl = sp.tile([128, 4, S], FP8, tag="atl")
        nc.vector.memset(vsb[:, :, :, 64:65], 1.0 / 16.0)

        # ================= phase 1: QKV + rope + transposes =================
        pq_pool = ExitStack()
        pp_qkv = pq_pool.enter_context(tc.tile_pool(name="pqkv", bufs=2, space="PSUM"))
        pp_tr = pq_pool.enter_context(tc.tile_pool(name="ptr", bufs=2, space="PSUM"))
        pre_pool = ExitStack()
        pp_pre = pre_pool.enter_context(tc.tile_pool(name="presc", bufs=1, space="PSUM"))

        pbs = [None] * NH
        pre_pbs = {}
        prescored = set()

        def score_group_thunks(it, h, pool, scbufs, paired):
            """Allocate pb for (it, h), return one thunk per psum score group
            (off-diag pair or diag single). Executing a thunk emits the
            matmul(s) + exp for that group."""
            kv = h // 4
            nkb = 4 * it + 4
            i0 = it * 512
            tag = "probs0" if (it == 0 and NTT >= 16) else "probs"
            pbufs = 8 if tag == "probs0" else 2
            pb = pr.tile([128, nkb, 512], BF16, tag=tag, bufs=pbufs,
                         name=f"pb{it}_{h}")
            pbs[h] = pb
            pre_pbs[(it, h)] = pb
            ndiag = 4 * it
            if paired:
                groups = [(kb, kb + 1) for kb in range(0, ndiag, 2)]
                groups += [(kb,) for kb in range(ndiag, nkb)]
            else:
                groups = [(kb,) for kb in range(nkb)]

            def emit_group(grp):
                if len(grp) == 2:
                    # off-diagonal pair: 2 one-bank psum slots, one wide exp
                    p2 = pool.tile([128, 2, 512], F32, tag="sc2", bufs=scbufs,
                                   name=f"psc{it}_{h}_{grp[0]}p")
                    for j, kb in enumerate(grp):
                        nc.tensor.matmul(p2[:, j, :],
                                         kt[:, kv, kb * 128:(kb + 1) * 128],
                                         qt[:, h, i0:i0 + 512],
                                         start=True, stop=True)
                    nc.scalar.activation(pb[:, grp[0]:grp[0] + 2, :], p2[:],
                                         EXP, scale=0.125, bias=nbias[:])
                    return
                kb = grp[0]
                diag = kb >= ndiag
                qlo = (kb - ndiag) * 128 if diag else 0
                psc = pool.tile([128, 512], F32, tag="sc2", bufs=scbufs,
                                name=f"psc{it}_{h}_{kb}")
                nc.tensor.matmul(psc[:, qlo:512],
                                 kt[:, kv, kb * 128:(kb + 1) * 128],
                                 qt[:, h, i0 + qlo:i0 + 512],
                                 start=True, stop=not diag)
                if diag:
                    nc.tensor.matmul(psc[:, qlo:qlo + 128], idb[:], trib[:],
                                     start=False, stop=True)
                nc.scalar.activation(pb[:, kb, qlo:512], psc[:, qlo:512],
                                     EXP, scale=0.125, bias=nbias[:])
                if qlo:
                    nc.gpsimd.memset(pb[:, kb, 0:qlo], 0.0)

            return [lambda grp=grp: emit_group(grp) for grp in groups]

        def emit_scores(it, h, pool, scbufs, paired=False):
            for t in score_group_thunks(it, h, pool, scbufs, paired):
                t()

        def p1_tail(tt, qk8):
            tsl = slice(tt * 128, (tt + 1) * 128)
            qtr = pp_tr.tile([64, 8, 128], BF16, tag="qtr", name=f"qtr{tt}")
            ktr = pp_tr.tile([64, 8, 128], BF16, tag="qtr", name=f"ktr{tt}")
            ktr = ktr[:, 0:2, :]
            for h in range(8):
                nc.tensor.matmul(qtr[:, h, :], qk8[:, h * 64:(h + 1) * 64],
                                 idb[:], is_transpose=True,
                                 start=(h == 0), stop=(h == 7))
            for g in range(2):
                nc.tensor.matmul(ktr[:, g, :],
                                 qk8[:, 512 + g * 64:512 + (g + 1) * 64],
                                 idb[:], is_transpose=True,
                                 start=(g == 0), stop=(g == 1))
            nc.vector.tensor_copy(qt[:, :, tsl], qtr[:])
            nc.vector.tensor_copy(kt[:, :, tsl], ktr[:])

        # Tiles 13..15 are deferred into phase 2 (it=0 is fully prescored, so
        # the PE sits under ACT-idle there; their psum comes from the po ring).
        NDEF = 3 if NTT >= 16 else 0
        NP1 = NTT - NDEF
        PRE = {5: [(0, 0)], 6: [(0, 1)], 7: [(0, 2)], 8: [(0, 3)],
               9: [(0, 4)], 10: [(0, 5)], 11: [(0, 6)], 12: [(0, 7)]}

        def emit_rope(tt, qkvb):
            # rope: op1 tmp = pairswap(qk)*sinsg (Pool); op2 t1 = qk*cos
            # (Pool); op3 qk8 = t1+tmp in-place (DVE, bf16 2x); + vsb copy
            qkv = qkvb[:, 0:640]
            swp = qkv.rearrange("p (h n two) -> p h n two", two=2, n=32)[..., ::-1]
            tmp = rp.tile([128, 640], BF16, tag="tmp", name=f"tmp{tt}")
            t1 = rp.tile([128, 640], BF16, tag="t1", name=f"t1_{tt}")
            sin4 = sin_sb[:, tt, :].rearrange("p (one n two) -> p one n two",
                                              one=1, two=2).to_broadcast([128, 10, 32, 2])
            cos3 = cos_sb[:, tt, :].rearrange("p (one c) -> p one c",
                                              one=1).to_broadcast([128, 10, 64])
            nc.vector.tensor_mul(tmp[:].rearrange("p (h n two) -> p h n two",
                                                  two=2, n=32),
                                 swp, sin4)
            nc.vector.tensor_mul(t1[:].rearrange("p (h c) -> p h c", h=10),
                                 qkv.rearrange("p (h c) -> p h c", h=10), cos3)
            nc.vector.tensor_add(t1[:], t1[:], tmp[:])
            nc.vector.tensor_copy(vsb[:, :, tt, 0:64],
                                  qkvb[:, 640:768].rearrange("p (kv c) -> p kv c",
                                                             kv=2))
            return t1

        prevq = []
        def qkv_kp(pq, xtile, kp):
            # exact-ish fp8 hi/lo split: (xh+xl)(wh+wl) ~ xh wh + xl wh + xh wl
            # per chunk-pair: 3 DoubleRow matmuls (0.75x bf16 cost); xl*wl
            # dropped
            xh = xtile[:, 2 * kp:2 * kp + 2, 0, :]
            xl = xtile[:, 2 * kp:2 * kp + 2, 1, :]
            for c0, c1 in ((0, 512), (512, 768)):
                wh = wall[:, 2 * kp:2 * kp + 2, 0, c0:c1]
                wl = wall[:, 2 * kp:2 * kp + 2, 1, c0:c1]
                nc.tensor.matmul(pq[:, c0:c1], xh, wh, start=(kp == 0),
                                 stop=False, perf_mode=DR)
                nc.tensor.matmul(pq[:, c0:c1], xl, wh, start=False, stop=False,
                                 perf_mode=DR)
                nc.tensor.matmul(pq[:, c0:c1], xh, wl, start=False,
                                 stop=(kp == KC // 2 - 1), perf_mode=DR)

        for tt in range(NP1):
            if tt + 2 < NTT:
                xfetch(tt + 2)
            if tt in (3, 5, 7, 9):
                w4 = tt // 2 - 1
                nc.sync.dma_start(woth[:, w4, :], wothb[:, w4, :])
            elif tt in (4, 6, 8, 10):
                w4 = tt // 2 - 2
                nc.sync.dma_start(wotl[:, w4, :], wotlb[:, w4, :])
            if tt == 0:
                # tiles 0+1 kp-major: QKV tracks the per-chunk wall DMA
                # arrivals instead of stalling tile 0 on each chunk
                xt0, xt1 = xtiles.pop(0), xtiles[1]
                pq0 = pp_qkv.tile([128, 1024], F32, tag="pq", name="pq0")
                pq1 = pp_qkv.tile([128, 1024], F32, tag="pq", name="pq1")
                for kp in range(KC // 2):
                    qkv_kp(pq0, xt0, kp)
                    qkv_kp(pq1, xt1, kp)
                pqs01 = [pq0, pq1]
                pq = pq0
            elif tt == 1:
                xtiles.pop(1)
                pq = pqs01[1]
            else:
                xtile = xtiles.pop(tt)
                pq = pp_qkv.tile([128, 1024], F32, tag="pq", name=f"pq{tt}")
                for kp in range(KC // 2):
                    qkv_kp(pq, xtile, kp)
            if len(prevq) >= 2:
                p1_tail(*prevq.pop(0))
            if NTT >= 16:
                for pit, ph in PRE.get(tt, ()):
                    emit_scores(pit, ph, pp_pre, 2)
                    prescored.add((pit, ph))
            qkvb = rp.tile([128, 768], BF16, tag="qkvb", name=f"qkvb{tt}")
            nc.scalar.activation(qkvb[:], pq[:, 0:768],
                                 mybir.ActivationFunctionType.Copy,
                                 scale=1.0 / 64.0)
            qk8 = emit_rope(tt, qkvb)
            prevq.append((tt, qk8))
        for pv in prevq:
            p1_tail(*pv)
        xfetch(NTT - 1)
        pre_pool.close()
        pq_pool.close()

        # ================= phase 2+3: attention + o-proj =================
        pp_att = ctx.enter_context(tc.tile_pool(name="patt", bufs=1, space="PSUM"))
        avs = [None] * 2
        atbs = {}
        atqs = {}
        oproj_q = []

        # deferred tiles: QKV in two po-ring halves + rope + transposes,
        # run as it=0 filler
        dqkvbs = {}
        dqk8s = {}

        def deferred_thunks(tt):
            def qkv_half(half):
                def f():
                    c0, c1 = (0, 512) if half == 0 else (512, 768)
                    xtile = xtiles[tt]
                    dpq = pp_att.tile([128, c1 - c0], F32, tag="po", bufs=2,
                                      name=f"dpq{tt}_{half}")
                    for kp in range(KC // 2):
                        xh = xtile[:, 2 * kp:2 * kp + 2, 0, :]
                        xl = xtile[:, 2 * kp:2 * kp + 2, 1, :]
                        wh = wall[:, 2 * kp:2 * kp + 2, 0, c0:c1]
                        wl = wall[:, 2 * kp:2 * kp + 2, 1, c0:c1]
                        nc.tensor.matmul(dpq[:], xh, wh, start=(kp == 0),
                                         stop=False, perf_mode=DR)
                        nc.tensor.matmul(dpq[:], xl, wh, start=False,
                                         stop=False, perf_mode=DR)
                        nc.tensor.matmul(dpq[:], xh, wl, start=False,
                                         stop=(kp == KC // 2 - 1), perf_mode=DR)
                    if half == 0:
                        dqkvbs[tt] = rp.tile([128, 768], BF16, tag="qkvb",
                                             name=f"qkvb{tt}")
                    else:
                        xtiles.pop(tt)
                    nc.scalar.activation(dqkvbs[tt][:, c0:c1], dpq[:],
                                         mybir.ActivationFunctionType.Copy,
                                         scale=1.0 / 64.0)
                return f

            def rope_f():
                dqk8s[tt] = emit_rope(tt, dqkvbs.pop(tt))

            def tail_f():
                tsl = slice(tt * 128, (tt + 1) * 128)
                qk8 = dqk8s.pop(tt)
                qtr = pp_att.tile([64, 8, 128], BF16, tag="po", bufs=2,
                                  name=f"dqtr{tt}")
                for h in range(8):
                    nc.tensor.matmul(qtr[:, h, :], qk8[:, h * 64:(h + 1) * 64],
                                     idb[:], is_transpose=True,
                                     start=(h == 0), stop=(h == 7))
                nc.vector.tensor_copy(qt[:, :, tsl], qtr[:])
                ktr = pp_att.tile([64, 8, 128], BF16, tag="po", bufs=2,
                                  name=f"dktr{tt}")
                for g in range(2):
                    nc.tensor.matmul(ktr[:, g, :],
                                     qk8[:, 512 + g * 64:512 + (g + 1) * 64],
                                     idb[:], is_transpose=True,
                                     start=(g == 0), stop=(g == 1))
                nc.vector.tensor_copy(kt[:, :, tsl], ktr[:, 0:2, :])

            return [qkv_half(0), qkv_half(1), rope_f, tail_f]

        deferred = []
        for dtt in range(NP1, NTT):
            deferred += deferred_thunks(dtt)

        def drain_thunk():
            if oproj_q:
                oproj_q.pop(0)()

        carry = [drain_thunk, drain_thunk]
        # it order: ACT-heavy its first (deferred tiles + drains fill under
        # them), prescored/ACT-free its last (absorb the o-proj drains).
        IT_ORDER = [2, 3, 1, 0] if NIT == 4 else list(range(NIT))
        for it in IT_ORDER:
            i0 = it * 512

            def av_thunks(it, h):
                kv = h // 4
                pb = pbs[h]
                out = []
                for tq in range(4):
                    def tqf(tq=tq, h=h, kv=kv, pb=pb, it=it):
                        tt = 4 * it + tq
                        if h % 2 == 0 and tq % 2 == 0:
                            avs[tq // 2] = pp_att.tile([128, 2, 2, 128], F32,
                                                       tag=f"avs{tq // 2}",
                                                       bufs=1,
                                                       name=f"avs{it}_{h}_{tq}")
                        slab = avs[tq // 2]
                        for kb in range(tt + 1):
                            nc.tensor.matmul(slab[:, tq % 2, h % 2, 0:65],
                                             pb[:, kb, tq * 128:(tq + 1) * 128],
                                             vsb[:, kv, kb, :],
                                             start=(kb == 0), stop=(kb == tt),
                                             skip_group_check=True)
                        if h % 2 == 1 and tq % 2 == 1:
                            # whole slab (2 tq x 2 h) complete: one fused DVE
                            # normalize for the pair (attr transposes later)
                            g = h // 2
                            sl2 = avs[tq // 2]
                            rec = sm.tile([128, 2, 2, 1], F32, tag="rec")
                            nc.vector.reciprocal(rec[:, :, :, 0],
                                                 sl2[:, :, :, 64])
                            atq = aq.tile([128, 2, 2, 64], BF16, tag="atq",
                                          name=f"atq{it}_{g}_{tq}")
                            nc.vector.tensor_mul(
                                atq[:], sl2[:, :, :, 0:64],
                                rec[:].to_broadcast([128, 2, 2, 64]))
                            atqs[(g, tq // 2)] = atq
                    out.append(tqf)
                return out

            def norm_thunks(it, g):
                out = []
                for tq in range(4):
                    def tqf(tq=tq, g=g, it=it):
                        if it not in atbs:
                            atbs[it] = aq.tile([128, 4, 512], BF16, tag="atb",
                                               bufs=1, name=f"atb{it}")
                        atq2 = atqs[(g, tq // 2)]
                        if tq % 2 == 1:
                            atqs.pop((g, tq // 2))
                        attr = pp_att.tile([128, 128], BF16, tag="po", bufs=2,
                                           name=f"attr{it}_{g}_{tq}")
                        nc.tensor.matmul(attr[:],
                                         atq2[:, tq % 2].rearrange(
                                             "p f c -> p (f c)"),
                                         idb[:], is_transpose=True,
                                         start=True, stop=True)
                        nc.vector.tensor_copy(
                            atbs[it][:, g, tq * 128:(tq + 1) * 128], attr[:])
                    out.append(tqf)

                def split_chunk(g=g, it=it):
                    # atb chunk (bf16, 16*at) -> ath/atl fp8 for DR o-proj
                    tsl = slice(it * 512, (it + 1) * 512)
                    atb = atbs[it]
                    nc.gpsimd.tensor_copy(ath[:, g, tsl], atb[:, g, :])
                    rtmp = aq.tile([128, 512], BF16, tag="rtmp", bufs=2,
                                   name=f"rtmp{it}_{g}")
                    nc.vector.tensor_sub(rtmp[:], atb[:, g, :], ath[:, g, tsl])
                    nc.gpsimd.tensor_copy(atl[:, g, tsl], rtmp[:])
                out.append(split_chunk)
                return out

            def queue_oproj_thunk(it_):
                # drains for its processed late run in the ACT-idle tail:
                # route their psum evictions to ACT, keeping DVE (tail
                # bottleneck) free; early its evict on DVE (ACT saturated).
                act_evict = it_ in (1, 0)

                def qt():
                    state = {}
                    for tq in range(4):
                        tt = 4 * it_ + tq
                        for nt in range(4):
                            def step(tt=tt, nt=nt):
                                tsl = slice(tt * 128, (tt + 1) * 128)
                                if nt == 0:
                                    state[tt] = ob.tile([128, D], BF16,
                                                        tag="osb",
                                                        name=f"osb{tt}")
                                osb = state[tt]
                                nsl = slice(nt * 512, (nt + 1) * 512)
                                po = pp_att.tile([128, 512], F32, tag="po",
                                                 bufs=2, name=f"po{tt}_{nt}")
                                k = 0
                                for c2 in range(2):
                                    cs = slice(2 * c2, 2 * c2 + 2)
                                    for a, w in ((ath, woth), (atl, woth),
                                                 (ath, wotl)):
                                        nc.tensor.matmul(
                                            po[:], a[:, cs, tsl], w[:, cs, nsl],
                                            start=(k == 0), stop=(k == 5),
                                            perf_mode=DR)
                                        k += 1
                                if act_evict:
                                    nc.scalar.activation(
                                        osb[:, nsl], po[:],
                                        mybir.ActivationFunctionType.Copy,
                                        scale=1.0 / 1024.0)
                                else:
                                    nc.vector.tensor_scalar_mul(
                                        osb[:, nsl], po[:], 1.0 / 1024.0)
                                nc.sync.dma_start(o[tsl, nsl], osb[:, nsl])
                            oproj_q.append(step)
                return qt

            for h in range(NH):
                # filler: PE work whose results ACT doesn't gate — runs
                # between score groups so exp (2.3x slower than the score
                # matmul) keeps up without stalling PE on the psum ring.
                filler = list(carry)
                carry = []
                if h > 0:
                    filler += av_thunks(it, h - 1)
                    filler += [drain_thunk]
                    if h % 2 == 0:
                        filler += norm_thunks(it, h // 2 - 1)
                    filler += [drain_thunk]
                    if h in (1, 2):
                        filler += [drain_thunk]
                if deferred:
                    take = 2 if h > 0 else 4
                    filler += deferred[:take]
                    del deferred[:take]
                if (it, h) in prescored:
                    pbs[h] = pre_pbs[(it, h)]
                    sc = []
                else:
                    sc = score_group_thunks(it, h, pp_att, 2, paired=True)
                # weighted riffle: spread filler evenly across score groups
                # (exp is ~2.3x slower than the score matmul; filler keeps PE
                # fed while ACT catches up)
                if not sc:
                    for t in filler:
                        t()
                else:
                    fi = 0
                    for i, t in enumerate(sc):
                        t()
                        want = (len(filler) * (i + 1)) // len(sc)
                        while fi < want:
                            filler[fi]()
                            fi += 1
                    while fi < len(filler):
                        filler[fi]()
                        fi += 1
            carry = av_thunks(it, NH - 1) + [drain_thunk]
            carry += norm_thunks(it, NH // 2 - 1)
            carry += [queue_oproj_thunk(it)]
        for t in carry:
            t()
        while oproj_q:
            oproj_q.pop(0)()

        if dbg:
            dsc = ctx.enter_context(tc.tile_pool(name="dsc", bufs=1))
            for nm, t in [("d_qt", qt[:]), ("d_kt", kt[:]), ("d_vsb", vsb[:]),
                          ("d_at", ath[:]), ("d_pb", pbs[0][:])]:
                f = dsc.tile(list(t.shape), F32, tag="f" + nm, name="f" + nm)
                nc.vector.tensor_copy(f[:], t)
                nc.sync.dma_start(dbg_d[nm][tuple(slice(None) for _ in t.shape)], f[:])
    nc.compile()
    return nc


# ====================== host side ======================

def _fp8(x):
    return np.asarray(x, np.float32).astype(ml_dtypes.float8_e4m3)


def host_inputs(x, Wq, Wk, Wv, Wo, S=2048):
    NTT = S // 128
    KC = D // 128
    inv = ROPE_BASE ** (-np.arange(0, DH, 2, dtype=np.float64) / DH)
    th = np.arange(S, dtype=np.float64)[:, None] * inv[None, :]
    cos1 = np.repeat(np.cos(th), 2, axis=1)
    sin1 = np.sin(th)
    sinsg1 = np.empty((S, 64))
    sinsg1[:, 0::2] = -sin1
    sinsg1[:, 1::2] = sin1
    cosb = np.ascontiguousarray(
        cos1.reshape(NTT, 128, 64).transpose(1, 0, 2)).astype(ml_dtypes.bfloat16)
    sing = np.ascontiguousarray(
        sinsg1.reshape(NTT, 128, 64).transpose(1, 0, 2)).astype(ml_dtypes.bfloat16)
    identf = np.eye(128, dtype=np.float32)
    p = np.arange(128)[:, None]
    q = np.arange(128)[None, :]
    trif = np.where(p <= q, 0.0, MASKVAL).astype(np.float32)

    in_maps = []
    for c in range(NCORES):
        b, kvp = c // 4, c % 4
        xb = np.asarray(x[b], np.float32)
        xh = _fp8(xb)
        xl = _fp8(xb - xh.astype(np.float32))
        xt = np.stack([xh, xl], axis=0).reshape(2, NTT, 128, KC, 128)
        xt8 = np.ascontiguousarray(xt.transpose(1, 4, 3, 0, 2))
        wq = Wq[512 * kvp:512 * (kvp + 1)]
        wk = Wk[128 * kvp:128 * (kvp + 1)]
        wv = Wv[128 * kvp:128 * (kvp + 1)]
        wall = np.concatenate([wq, wk, wv], axis=0) * 64.0
        wh = _fp8(wall)
        wl = _fp8(wall - wh.astype(np.float32))
        wall8 = np.ascontiguousarray(
            np.stack([wh, wl], axis=0).transpose(2, 0, 1)
            .reshape(KC, 128, 2, 768).transpose(1, 0, 2, 3))
        wo64 = np.ascontiguousarray(
            (Wo[:, 512 * kvp:512 * (kvp + 1)] * 64.0).astype(np.float32)
            .T.reshape(4, 128, D).transpose(1, 0, 2))
        woh = _fp8(wo64)
        wol = _fp8(wo64 - woh.astype(np.float32))
        in_maps.append(dict(xt8=xt8, wall8=wall8, wothb=woh, wotlb=wol,
                            cosb=cosb, sinsg=sing, identf=identf, trif=trif))
    return in_maps


def kernel(**inputs):
    x = np.asarray(inputs["x"], dtype=np.float32)
    Wq = np.asarray(inputs["Wq"], dtype=np.float32)
    Wk = np.asarray(inputs["Wk"], dtype=np.float32)
    Wv = np.asarray(inputs["Wv"], dtype=np.float32)
    Wo = np.asarray(inputs["Wo"], dtype=np.float32)
    B, S, _ = x.shape
    in_maps = host_inputs(x, Wq, Wk, Wv, Wo, S=S)
    if "nc" not in _cached:
        _cached["nc"] = build_nc(S=S)
    res = run_bass_kernel_spmd(_cached["nc"], in_maps, list(range(NCORES)))
    out = np.zeros((B, S, D), np.float64)
    for c, r in enumerate(res.results):
        out[c // 4] += np.asarray(r["o"], np.float32)
    return out.astype(np.float32)

